# revision 4
# baseline (speedup 1.0000x reference)
# Trainium2 Bass kernel for the DVAE encoder (nn_DVAE_24850680775463).
#
# Sharding: pure data-parallel. B=1024 graphs -> 8 cores x 128 graphs.
# Per core, the 128 graphs sit on the 128 SBUF partitions and the whole
# 16-vertex sequential scan runs on-chip.
#
# Key restructurings vs the reference:
#  * gate/mapper products are computed once per vertex (incremental cache)
#    instead of for all 16 candidate predecessors every step; the vertex-id
#    one-hot contribution folds into a per-vertex bias row.
#  * the adjacency-weighted message H_v = sum_n adj[b,v,n] * gm[b,n,:] is
#    computed on the TensorEngine as PSUM-accumulated matmuls with
#    diag(adj[:,v,n]) as the stationary operand.
#  * small-K matmuls (one-hot x-side, bias rows) are packed into 32-row
#    PE array strips via tile_position so up to 4 run concurrently.
#  * transposes are chunk-pipelined (4x128 PE transposes + per-chunk DVE
#    copies) so dependent matmuls start as soon as their chunk lands.
#  * off-critical elementwise (diag builds, 1-z, z*h) runs on the Pool
#    engine, which is otherwise idle.
# Matmuls default to bf16 (fp32 PSUM accumulation).

import os
import numpy as np

import concourse.bass as bass
import concourse.tile as tile
from concourse import bacc, mybir
from concourse.bass_utils import run_bass_kernel_spmd

AF = mybir.ActivationFunctionType
F32 = mybir.dt.float32

NCORES = 8
B, NV, NVT, FS, HS, NZ = 1024, 16, 16, 32, 512, 64
P = B // NCORES            # 128 graphs per core
G3 = 3 * HS                # 1536
K1 = NVT + 1               # 17  (one-hot + ones row)
K2 = FS + 1                # 33  (params + ones row)
KC = HS // 128             # 4 contraction chunks of the hidden dim

MMDT = {"f32r": mybir.dt.float32r, "f32": mybir.dt.float32,
        "bf16": mybir.dt.bfloat16}[os.environ.get("DVAE_MMDT", "bf16")]

FILL1 = int(os.environ.get("DVAE_FILL1", "10"))
FILL2 = int(os.environ.get("DVAE_FILL2", "10"))
FILL3 = int(os.environ.get("DVAE_FILL3", "8"))


def build_bass():
    nc = bacc.Bacc("TRN2", target_bir_lowering=False, debug=False)

    def inp(name, shape, dt=None):
        return nc.dram_tensor(name, shape, dt or MMDT,
                              kind="ExternalInput").ap()

    d = {
        "xt1p":  inp("xt1p",  [128, NV * P]),
        "w1p":   inp("w1p",   [128, HS]),
        "eyeb":  inp("eyeb",  [128, 128]),
        "adjt":  inp("adjt",  [P, NV * NV], F32),
        "xp1p":  inp("xp1p",  [128, NV * P]),
        "w2p":   inp("w2p",   [128, 2 * HS]),
        "wht_t": inp("wht_t", [128, KC * G3]),
        "wht_p": inp("wht_p", [128, KC * G3]),
        "wgm":   inp("wgm",   [128, KC * 2 * HS]),
        "vselp": inp("vselp", [128, NV * P]),
        "bgmp":  inp("bgmp",  [128, HS]),
        "wfc":   inp("wfc",   [128, KC * 2 * NZ]),
        "bfc":   inp("bfc",   [1, 2 * NZ]),
        "ones1": inp("ones1", [1, 128]),
    }
    out_ap = nc.dram_tensor("out", [P, 2 * NZ], mybir.dt.float32, kind="ExternalOutput").ap()

    with tile.TileContext(nc) as tc:
        _body(tc, d, out_ap)
    nc.compile()
    return nc


def _body(tc, d, out_ap):
    nc = tc.nc
    from contextlib import ExitStack
    with ExitStack() as ctx:
        wp = ctx.enter_context(tc.tile_pool(name="w", bufs=1))
        sp = ctx.enter_context(tc.tile_pool(name="s", bufs=1))
        dgp = ctx.enter_context(tc.tile_pool(name="dg", bufs=16))
        gmc = ctx.enter_context(tc.tile_pool(name="gmc", bufs=1))
        ps_h = ctx.enter_context(tc.tile_pool(name="psh", bufs=1, space="PSUM"))
        ps_g = ctx.enter_context(tc.tile_pool(name="psg", bufs=4, space="PSUM"))
        ps_m = ctx.enter_context(tc.tile_pool(name="psm", bufs=1, space="PSUM"))
        ps_d = ctx.enter_context(tc.tile_pool(name="psd", bufs=1, space="PSUM"))

        # ---- persistent weights / constants ----
        order = ["ones1", "xt1p", "w1p", "eyeb", "adjt", "xp1p", "w2p",
                 "wht_t", "wht_p", "wgm", "vselp", "bgmp", "wfc", "bfc"]
        W = {}
        for name, ap in sorted(d.items(), key=lambda kv: order.index(kv[0])):
            t = wp.tile(list(ap.shape), ap.dtype, tag=name)
            nc.sync.dma_start(t[:], ap[:, :])
            W[name] = t

        wht = {0: W["wht_t"], 1: W["wht_p"]}
        adjt = W["adjt"]
        eyeb = W["eyeb"]

        gm_sb = []          # cached gate*mapped per vertex, [P, HS] each

        def cs(c):
            return slice(c * 128, (c + 1) * 128)

        def transpose_chunks(src_sb, tag):
            """[128,512] batch-major -> feature-major, chunk-pipelined.
            Emits 4 PE transposes + 4 DVE chunk copies; dependent matmuls
            on chunk c only wait for chunk c's copy. The PSUM scratch
            shares gatep's bank (their lifetimes are disjoint)."""
            tp = ps_m.tile([128, HS], MMDT, tag="psm")
            dst = sp.tile([128, HS], MMDT, tag=tag)
            for c in range(KC):
                nc.tensor.transpose(tp[:, cs(c)], src_sb[:, cs(c)], eyeb[:])
                nc.vector.tensor_copy(dst[:, cs(c)], tp[:, cs(c)])
            return dst

        def xslot(g, v, R, Z, NI, NH, has_h):
            """Packed small-K matmuls: x-side of all gates + NH bias row,
            run concurrently in distinct 32-row PE array strips."""
            vs = slice(v * P, (v + 1) * P)
            st = not has_h
            if g == 0:
                x, w = W["xt1p"], W["w1p"]
                nc.tensor.matmul(R[:], x[0:K1, vs], w[0:K1, :],
                                 start=True, stop=st, tile_position=(0, 0))
                nc.tensor.matmul(Z[:], x[32:32 + K1, vs], w[32:32 + K1, :],
                                 start=True, stop=st, tile_position=(32, 0))
                nc.tensor.matmul(NI[:], x[64:64 + K1, vs], w[64:64 + K1, :],
                                 start=True, stop=True, tile_position=(64, 0))
                nc.tensor.matmul(NH[:], x[96:97, vs], w[96:97, :],
                                 start=True, stop=st, tile_position=(96, 0))
            else:
                x, w = W["xp1p"], W["w2p"]
                nc.tensor.matmul(R[:], x[0:K2, vs], w[0:K2, 0:HS],
                                 start=True, stop=st, tile_position=(0, 0))
                nc.tensor.matmul(Z[:], x[64:64 + K2, vs], w[64:64 + K2, 0:HS],
                                 start=True, stop=st, tile_position=(64, 0))
                nc.tensor.matmul(NI[:], x[0:K2, vs], w[0:K2, HS:2 * HS],
                                 start=True, stop=True, tile_position=(0, 0))
                nc.tensor.matmul(NH[:], x[96:97, vs], w[96:97, HS:2 * HS],
                                 start=True, stop=st, tile_position=(96, 0))

        def gru_hmms(g, R, Z, NH, HT):
            """h-side chunk matmuls; R first (its consumer leads the ew
            chain), then NH, then Z."""
            w = wht[g]
            for c in range(KC):
                nc.tensor.matmul(R[:], HT[:, cs(c)], w[:, c * G3:c * G3 + HS],
                                 start=False, stop=c == KC - 1)
            for c in range(KC):
                nc.tensor.matmul(NH[:], HT[:, cs(c)],
                                 w[:, c * G3 + 2 * HS:(c + 1) * G3],
                                 start=False, stop=c == KC - 1)
            for c in range(KC):
                nc.tensor.matmul(Z[:], HT[:, cs(c)],
                                 w[:, c * G3 + HS:c * G3 + 2 * HS],
                                 start=False, stop=c == KC - 1)

        def gru_ew(R, Z, NI, NH, h_sb):
            """Elementwise GRU combine. h_sb: batch-major h (MMDT) or None.
            Critical chain r->rhn->npre->tanh->zn->hv on Act/DVE; the z
            path (zc, zh) runs on the otherwise-idle Pool engine."""
            r = sp.tile([128, HS], MMDT, tag="r")
            nc.scalar.activation(r[:], R[:], AF.Sigmoid)
            z = zh = None
            if h_sb is not None:
                z = sp.tile([128, HS], MMDT, tag="z")
                nc.scalar.activation(z[:], Z[:], AF.Sigmoid)
            rhn = sp.tile([128, HS], MMDT, tag="rhn")
            nc.vector.tensor_mul(rhn[:], r[:], NH[:])
            npre = sp.tile([128, HS], MMDT, tag="npre")
            nc.vector.tensor_add(npre[:], rhn[:], NI[:])
            n = sp.tile([128, HS], MMDT, tag="n")
            nc.scalar.activation(n[:], npre[:], AF.Tanh)
            zc = sp.tile([128, HS], MMDT, tag="zc")
            if h_sb is not None:
                nc.gpsimd.tensor_scalar(zc[:], z[:], -1.0, 1.0,
                                        mybir.AluOpType.mult,
                                        mybir.AluOpType.add)
                zh = sp.tile([128, HS], MMDT, tag="zh")
                nc.gpsimd.tensor_mul(zh[:], z[:], h_sb[:])
            else:
                nc.scalar.activation(zc[:], Z[:], AF.Sigmoid, scale=-1.0)
            zn = sp.tile([128, HS], MMDT, tag="zn")
            nc.vector.tensor_mul(zn[:], zc[:], n[:])
            if h_sb is None:
                return zn
            hv = sp.tile([128, HS], MMDT, tag="hv")
            nc.vector.tensor_add(hv[:], zn[:], zh[:])
            return hv

        dum = ps_d.tile([128, 256], F32, tag="dum")

        def fill_pe(k):
            """k dependency-free matmuls into a scratch bank: keeps the PE
            HAM clock-gate warm across elementwise-chain gaps."""
            for _ in range(k):
                nc.tensor.matmul(dum[:], eyeb[:], W["wgm"][:, 0:256],
                                 start=True, stop=True)

        def diag_build(w, n):
            dg = dgp.tile([128, 128], MMDT, tag="diag")
            nc.gpsimd.tensor_scalar_mul(dg[:], eyeb[:],
                                        adjt[:, w * NV + n:w * NV + n + 1])
            return dg

        def diag_mm(Hp, dg, n, start, stop):
            nc.tensor.matmul(Hp[:], dg[:], gm_sb[n][:],
                             start=start, stop=stop)

        H_next = None

        for v in range(NV):
            # --- HT preparation (chunk-pipelined) ---
            if v == 0:
                hH = None
                HT = None
                R = ps_g.tile([128, HS], F32, tag="g")
                Z = ps_g.tile([128, HS], F32, tag="g")
                NI = ps_g.tile([128, HS], F32, tag="g")
                NH = ps_g.tile([128, HS], F32, tag="g")
                xslot(0, v, R, Z, NI, NH, has_h=False)
            else:
                R = ps_g.tile([128, HS], F32, tag="g")
                Z = ps_g.tile([128, HS], F32, tag="g")
                NI = ps_g.tile([128, HS], F32, tag="g")
                NH = ps_g.tile([128, HS], F32, tag="g")
                xslot(0, v, R, Z, NI, NH, has_h=True)
                hH = sp.tile([128, HS], MMDT, tag="hH")
                nc.scalar.copy(hH[:], H_next[:])
                HT = transpose_chunks(hH, "HT")
                gru_hmms(0, R, Z, NH, HT)

            # fill the GRU1-elementwise PE gap with next-step message terms
            avail = list(range(v))   # gm_0..gm_{v-1} exist already
            half = (len(avail) + 1) // 2
            if v < NV - 1:
                H_next = ps_h.tile([128, HS], F32, tag="H")
                dgs = {i: diag_build(v + 1, i) for i in range(v + 1)}
                for i in avail[:half]:
                    diag_mm(H_next, dgs[i], i, start=(i == 0), stop=False)
            # GRU-p x-slots: banks free up as the GRU-t ew chain consumes
            # them, so these run inside the ew window
            R2 = ps_g.tile([128, HS], F32, tag="g")
            Z2 = ps_g.tile([128, HS], F32, tag="g")
            NI2 = ps_g.tile([128, HS], F32, tag="g")
            NH2 = ps_g.tile([128, HS], F32, tag="g")
            xslot(1, v, R2, Z2, NI2, NH2, has_h=True)
            fill_pe(0 if v < 2 else max(0, FILL1 - 2 * len(avail[:half])))
            hv1 = gru_ew(R, Z, NI, NH, hH)
            hv1T = transpose_chunks(hv1, "hv1T")
            gru_hmms(1, R2, Z2, NH2, hv1T)
            if v < NV - 1:
                for i in avail[half:]:
                    diag_mm(H_next, dgs[i], i, start=(i == 0), stop=False)
            fill_pe(0 if v < 2 else max(0, FILL2 - 2 * len(avail[half:])))
            hv = gru_ew(R2, Z2, NI2, NH2, hv1)
            hvT = transpose_chunks(hv, "hvT")

            if v < NV - 1:
                # gate/mapper for vertex v (feeds future messages)
                gatep = ps_m.tile([128, HS], F32, tag="psm")
                mapp = ps_m.tile([128, HS], F32, tag="psmt")
                vs = slice(v * P, (v + 1) * P)
                nc.tensor.matmul(gatep[:], W["vselp"][0:NV, vs],
                                 W["bgmp"][0:NV, :],
                                 start=True, stop=False, tile_position=(0, 0))
                nc.tensor.matmul(mapp[:], W["vselp"][32:32 + NV, vs],
                                 W["bgmp"][32:32 + NV, :],
                                 start=True, stop=False, tile_position=(32, 0))
                # gate chunks first so the sigmoid can start while the
                # mapper chunks still stream
                for c in range(KC):
                    nc.tensor.matmul(gatep[:], hvT[:, cs(c)],
                                     W["wgm"][:, c * 2 * HS:c * 2 * HS + HS],
                                     start=False, stop=c == KC - 1)
                for c in range(KC):
                    nc.tensor.matmul(mapp[:], hvT[:, cs(c)],
                                     W["wgm"][:, c * 2 * HS + HS:(c + 1) * 2 * HS],
                                     start=False, stop=c == KC - 1)
                gate = sp.tile([128, HS], MMDT, tag="gate")
                nc.scalar.activation(gate[:], gatep[:], AF.Sigmoid)
                gmt = gmc.tile([128, HS], MMDT, tag=f"gm{v}")
                nc.vector.tensor_mul(gmt[:], gate[:], mapp[:])
                gm_sb.append(gmt)
                # last message term for step v+1 (needs gm_v)
                diag_mm(H_next, dgs[v], v, start=(v == 0), stop=True)
                fill_pe(0 if v < 2 else FILL3)
            else:
                # final FC: out = Hg @ Wfc + bfc   (mu | logvar)
                fcp = ps_m.tile([128, 2 * NZ], F32, tag="psm")
                nc.tensor.matmul(fcp[:], W["ones1"][:], W["bfc"][:, :],
                                 start=True, stop=False)
                for c in range(KC):
                    nc.tensor.matmul(fcp[:], hvT[:, cs(c)],
                                     W["wfc"][:, c * 2 * NZ:(c + 1) * 2 * NZ],
                                     start=False, stop=(c == KC - 1))
                fc = sp.tile([128, 2 * NZ], F32, tag="fc")
                nc.scalar.copy(fc[:], fcp[:])
                nc.sync.dma_start(out_ap[:, :], fc[:])


def _host_prep(types, params, adj, gt_wi, gt_wh, gt_bi, gt_bh,
               gp_wi, gp_wh, gp_bi, gp_bh, gate_w, gate_b, mapper_w,
               fc1_w, fc1_b, fc2_w, fc2_b):
    """Pure layout prep: transposes/reshapes/one-hot + per-core sharding."""
    f = np.float32

    def chunked(a):  # [512, X] -> [128, 4*X] with K-chunks side by side
        X = a.shape[1]
        return np.ascontiguousarray(
            a.reshape(KC, 128, X).transpose(1, 0, 2).reshape(128, KC * X)).astype(f)

    b1 = np.concatenate([(gt_bi + gt_bh)[:2 * HS], gt_bi[2 * HS:]])
    b2 = np.concatenate([(gp_bi + gp_bh)[:2 * HS], gp_bi[2 * HS:]])
    w1x = np.concatenate([gt_wi.T, b1[None, :]], 0).astype(f)   # [17, 1536]
    w2x = np.concatenate([gp_wi.T, b2[None, :]], 0).astype(f)   # [33, 1536]

    # packed strip layouts: gate columns of the x-side weights stacked at
    # 32-row offsets so the strip matmuls stream from matching partitions
    w1p = np.zeros((128, HS), f)
    w1p[0:K1] = w1x[:, 0:HS]            # R
    w1p[32:32 + K1] = w1x[:, HS:2 * HS]  # Z
    w1p[64:64 + K1] = w1x[:, 2 * HS:G3]  # NI
    w1p[96] = gt_bh[2 * HS:]             # NH bias
    w2p = np.zeros((128, 2 * HS), f)
    w2p[0:K2, 0:HS] = w2x[:, 0:HS]
    w2p[64:64 + K2, 0:HS] = w2x[:, HS:2 * HS]
    w2p[0:K2, HS:2 * HS] = w2x[:, 2 * HS:G3]
    w2p[96, HS:2 * HS] = gp_bh[2 * HS:]

    bgm = np.stack([np.concatenate([gate_b + gate_w[:, HS + v],
                                    mapper_w[:, HS + v]])
                    for v in range(NV)]).astype(f)               # [NV, 1024]
    bgmp = np.zeros((128, HS), f)
    bgmp[0:NV] = bgm[:, 0:HS]
    bgmp[32:32 + NV] = bgm[:, HS:2 * HS]
    vsel = np.repeat(np.eye(NV, dtype=f), P, axis=1)             # [NV, NV*P]
    vselp = np.zeros((128, NV * P), f)
    vselp[0:NV] = vsel
    vselp[32:32 + NV] = vsel

    shared = {
        "w1p": w1p, "w2p": w2p, "vselp": vselp, "bgmp": bgmp,
        "wht_t": chunked(gt_wh.T.astype(f)),
        "wht_p": chunked(gp_wh.T.astype(f)),
        "wgm": chunked(np.concatenate([gate_w[:, :HS].T, mapper_w[:, :HS].T], 1)),
        "wfc": chunked(np.concatenate([fc1_w.T, fc2_w.T], 1).astype(f)),
        "bfc": np.concatenate([fc1_b, fc2_b])[None, :].astype(f),
        "eyeb": np.eye(128, dtype=f),
        "ones1": np.ones((1, 128), f),
    }
    oh = (types[:, :, None] == np.arange(NVT)[None, None, :]).astype(f)  # [B,NV,NVT]
    in_maps = []
    for c in range(NCORES):
        s = slice(c * P, (c + 1) * P)
        xt = oh[s].transpose(2, 1, 0).reshape(NVT, NV * P)           # [16, NV*P]
        xt1 = np.concatenate([xt, np.ones((1, NV * P), f)], 0)       # [17, NV*P]
        xp = params[s].transpose(2, 1, 0).reshape(FS, NV * P).astype(f)
        xp1 = np.concatenate([xp, np.ones((1, NV * P), f)], 0)       # [33, NV*P]
        xt1p = np.zeros((128, NV * P), f)
        xt1p[0:K1] = xt1
        xt1p[32:32 + K1] = xt1
        xt1p[64:64 + K1] = xt1
        xt1p[96] = 1.0
        xp1p = np.zeros((128, NV * P), f)
        xp1p[0:K2] = xp1
        xp1p[64:64 + K2] = xp1          # row 96 = ones (bias stationary)
        m = dict(shared)
        m["xt1p"] = xt1p
        m["xp1p"] = xp1p
        m["adjt"] = np.ascontiguousarray(adj[s].reshape(P, NV * NV)).astype(f)
        in_maps.append(m)
    return in_maps


_NC_CACHE = {}


def _get_nc():
    key = str(MMDT)
    if key not in _NC_CACHE:
        _NC_CACHE[key] = build_bass()
    return _NC_CACHE[key]


F32_INPUTS = {"adjt"}


def kernel(**inputs):
    np_inputs = {k: np.asarray(v) for k, v in inputs.items()}
    in_maps = _host_prep(**np_inputs)
    npdt = mybir.dt.np(MMDT)
    if npdt != np.float32:
        in_maps = [{k: (v if k in F32_INPUTS else v.astype(npdt))
                    for k, v in m.items()} for m in in_maps]
    nc = _get_nc()
    res = run_bass_kernel_spmd(nc, in_maps, core_ids=list(range(NCORES)),
                               **_RUN_KWARGS)
    out = np.concatenate([res.results[c]["out"] for c in range(NCORES)], 0)
    _LAST_RESULT.clear()
    _LAST_RESULT.append(res)
    return out[:, :NZ], out[:, NZ:]


# test.py can set these to enable tracing / inspect results
_RUN_KWARGS = {}
_LAST_RESULT = []


# revision 5
# speedup vs baseline: 1.4064x; 1.4064x over previous
# Trainium2 Bass kernel for the DVAE encoder (nn_DVAE_24850680775463).
#
# Sharding: pure data-parallel. B=1024 graphs -> 8 cores x 128 graphs.
# Per core, the 128 graphs sit on the 128 SBUF partitions and the whole
# 16-vertex sequential scan runs on-chip.
#
# Key restructurings vs the reference:
#  * gate/mapper products are computed once per vertex (incremental cache)
#    instead of for all 16 candidate predecessors every step; the vertex-id
#    one-hot contribution folds into a per-vertex bias row.
#  * the adjacency-weighted message H_v = sum_n adj[b,v,n] * gm[b,n,:] is
#    computed on the TensorEngine as PSUM-accumulated matmuls with
#    diag(adj[:,v,n]) as the stationary operand.
#  * small-K matmuls (one-hot x-side, bias rows) are packed into 32-row
#    PE array strips via tile_position so up to 4 run concurrently.
#  * transposes are chunk-pipelined (4x128 PE transposes + per-chunk DVE
#    copies) so dependent matmuls start as soon as their chunk lands.
#  * off-critical elementwise (diag builds, 1-z, z*h) runs on the Pool
#    engine, which is otherwise idle.
# Matmuls default to bf16 (fp32 PSUM accumulation).

import os
import numpy as np

import concourse.bass as bass
import concourse.tile as tile
from concourse import bacc, mybir
from concourse.bass_utils import run_bass_kernel_spmd

AF = mybir.ActivationFunctionType
F32 = mybir.dt.float32

NCORES = 8
B, NV, NVT, FS, HS, NZ = 1024, 16, 16, 32, 512, 64
P = B // NCORES            # 128 graphs per core
G3 = 3 * HS                # 1536
K1 = NVT + 1               # 17  (one-hot + ones row)
K2 = FS + 1                # 33  (params + ones row)
KC = HS // 128             # 4 contraction chunks of the hidden dim

MMDT = {"f32r": mybir.dt.float32r, "f32": mybir.dt.float32,
        "bf16": mybir.dt.bfloat16}[os.environ.get("DVAE_MMDT", "bf16")]

FILL1 = int(os.environ.get("DVAE_FILL1", "22"))
FILL2 = int(os.environ.get("DVAE_FILL2", "22"))
FILL3 = int(os.environ.get("DVAE_FILL3", "16"))


def build_bass():
    nc = bacc.Bacc("TRN2", target_bir_lowering=False, debug=False)

    def inp(name, shape, dt=None):
        return nc.dram_tensor(name, shape, dt or MMDT,
                              kind="ExternalInput").ap()

    d = {
        "xt1p":  inp("xt1p",  [128, NV * P]),
        "w1p":   inp("w1p",   [128, HS]),
        "eyeb":  inp("eyeb",  [128, 128]),
        "adjt":  inp("adjt",  [P, NV * NV], F32),
        "xp1p":  inp("xp1p",  [128, NV * P]),
        "w2p":   inp("w2p",   [128, 2 * HS]),
        "wht_t": inp("wht_t", [128, KC * G3]),
        "wht_p": inp("wht_p", [128, KC * G3]),
        "wgm":   inp("wgm",   [128, KC * 2 * HS]),
        "vselp": inp("vselp", [128, NV * P]),
        "bgmp":  inp("bgmp",  [128, HS]),
        "wfc":   inp("wfc",   [128, KC * 2 * NZ]),
        "bfc":   inp("bfc",   [1, 2 * NZ]),
        "ones1": inp("ones1", [1, 128]),
    }
    out_ap = nc.dram_tensor("out", [P, 2 * NZ], mybir.dt.float32, kind="ExternalOutput").ap()

    with tile.TileContext(nc) as tc:
        _body(tc, d, out_ap)
    nc.compile()
    return nc


def _body(tc, d, out_ap):
    nc = tc.nc
    from contextlib import ExitStack
    with ExitStack() as ctx:
        wp = ctx.enter_context(tc.tile_pool(name="w", bufs=1))
        sp = ctx.enter_context(tc.tile_pool(name="s", bufs=1))
        dgp = ctx.enter_context(tc.tile_pool(name="dg", bufs=16))
        gmc = ctx.enter_context(tc.tile_pool(name="gmc", bufs=1))
        ps_h = ctx.enter_context(tc.tile_pool(name="psh", bufs=1, space="PSUM"))
        ps_g = ctx.enter_context(tc.tile_pool(name="psg", bufs=4, space="PSUM"))
        ps_m = ctx.enter_context(tc.tile_pool(name="psm", bufs=1, space="PSUM"))
        ps_d = ctx.enter_context(tc.tile_pool(name="psd", bufs=1, space="PSUM"))

        # ---- persistent weights / constants ----
        order = ["ones1", "xt1p", "w1p", "eyeb", "adjt", "xp1p", "w2p",
                 "wht_t", "wht_p", "wgm", "vselp", "bgmp", "wfc", "bfc"]
        W = {}
        for name, ap in sorted(d.items(), key=lambda kv: order.index(kv[0])):
            t = wp.tile(list(ap.shape), ap.dtype, tag=name)
            nc.sync.dma_start(t[:], ap[:, :])
            W[name] = t

        wht = {0: W["wht_t"], 1: W["wht_p"]}
        adjt = W["adjt"]
        eyeb = W["eyeb"]

        gm_sb = []          # cached gate*mapped per vertex, [P, HS] each

        def cs(c):
            return slice(c * 128, (c + 1) * 128)

        def transpose_chunks(src_sb, tag):
            """[128,512] batch-major -> feature-major, chunk-pipelined.
            Emits 4 PE transposes + 4 DVE chunk copies; dependent matmuls
            on chunk c only wait for chunk c's copy. The PSUM scratch
            shares gatep's bank (their lifetimes are disjoint)."""
            tp = ps_m.tile([128, HS], MMDT, tag="psm")
            dst = sp.tile([128, HS], MMDT, tag=tag)
            for c in range(KC):
                nc.tensor.transpose(tp[:, cs(c)], src_sb[:, cs(c)], eyeb[:])
                nc.vector.tensor_copy(dst[:, cs(c)], tp[:, cs(c)])
            return dst

        def xslot(g, v, R, Z, NI, NH, has_h):
            """Packed small-K matmuls: x-side of all gates + NH bias row,
            run concurrently in distinct 32-row PE array strips."""
            vs = slice(v * P, (v + 1) * P)
            st = not has_h
            if g == 0:
                x, w = W["xt1p"], W["w1p"]
                nc.tensor.matmul(R[:], x[0:K1, vs], w[0:K1, :],
                                 start=True, stop=st, tile_position=(0, 0))
                nc.tensor.matmul(Z[:], x[32:32 + K1, vs], w[32:32 + K1, :],
                                 start=True, stop=st, tile_position=(32, 0))
                nc.tensor.matmul(NI[:], x[64:64 + K1, vs], w[64:64 + K1, :],
                                 start=True, stop=True, tile_position=(64, 0))
                nc.tensor.matmul(NH[:], x[96:97, vs], w[96:97, :],
                                 start=True, stop=st, tile_position=(96, 0))
            else:
                x, w = W["xp1p"], W["w2p"]
                nc.tensor.matmul(R[:], x[0:K2, vs], w[0:K2, 0:HS],
                                 start=True, stop=st, tile_position=(0, 0))
                nc.tensor.matmul(Z[:], x[64:64 + K2, vs], w[64:64 + K2, 0:HS],
                                 start=True, stop=st, tile_position=(64, 0))
                nc.tensor.matmul(NI[:], x[0:K2, vs], w[0:K2, HS:2 * HS],
                                 start=True, stop=True, tile_position=(0, 0))
                nc.tensor.matmul(NH[:], x[96:97, vs], w[96:97, HS:2 * HS],
                                 start=True, stop=st, tile_position=(96, 0))

        def gru_hmms(g, R, Z, NH, HT):
            """h-side chunk matmuls; R first (its consumer leads the ew
            chain), then NH, then Z."""
            w = wht[g]
            for c in range(KC):
                nc.tensor.matmul(R[:], HT[:, cs(c)], w[:, c * G3:c * G3 + HS],
                                 start=False, stop=c == KC - 1)
            for c in range(KC):
                nc.tensor.matmul(NH[:], HT[:, cs(c)],
                                 w[:, c * G3 + 2 * HS:(c + 1) * G3],
                                 start=False, stop=c == KC - 1)
            for c in range(KC):
                nc.tensor.matmul(Z[:], HT[:, cs(c)],
                                 w[:, c * G3 + HS:c * G3 + 2 * HS],
                                 start=False, stop=c == KC - 1)

        def gru_ew(R, Z, NI, NH, h_sb):
            """Elementwise GRU combine. h_sb: batch-major h (MMDT) or None.
            Critical chain r->rhn->npre->tanh->zn->hv on Act/DVE; the z
            path (zc, zh) runs on the otherwise-idle Pool engine."""
            r = sp.tile([128, HS], MMDT, tag="r")
            nc.scalar.activation(r[:], R[:], AF.Sigmoid)
            z = zh = None
            if h_sb is not None:
                z = sp.tile([128, HS], MMDT, tag="z")
                nc.scalar.activation(z[:], Z[:], AF.Sigmoid)
            rhn = sp.tile([128, HS], MMDT, tag="rhn")
            nc.vector.tensor_mul(rhn[:], r[:], NH[:])
            npre = sp.tile([128, HS], MMDT, tag="npre")
            nc.vector.tensor_add(npre[:], rhn[:], NI[:])
            n = sp.tile([128, HS], MMDT, tag="n")
            nc.scalar.activation(n[:], npre[:], AF.Tanh)
            zc = sp.tile([128, HS], MMDT, tag="zc")
            if h_sb is not None:
                nc.vector.tensor_scalar(zc[:], z[:], -1.0, 1.0,
                                        mybir.AluOpType.mult,
                                        mybir.AluOpType.add)
                zh = sp.tile([128, HS], MMDT, tag="zh")
                nc.vector.tensor_mul(zh[:], z[:], h_sb[:])
            else:
                nc.scalar.activation(zc[:], Z[:], AF.Sigmoid, scale=-1.0)
            zn = sp.tile([128, HS], MMDT, tag="zn")
            nc.vector.tensor_mul(zn[:], zc[:], n[:])
            if h_sb is None:
                return zn
            hv = sp.tile([128, HS], MMDT, tag="hv")
            nc.vector.tensor_add(hv[:], zn[:], zh[:])
            return hv

        dum = ps_d.tile([128, 256], F32, tag="dum")

        def fill_pe(k):
            """k dependency-free matmuls into a scratch bank: keeps the PE
            HAM clock-gate warm across elementwise-chain gaps."""
            for _ in range(k):
                nc.tensor.matmul(dum[:], eyeb[:], W["wgm"][:, 0:256],
                                 start=True, stop=True)

        def diag_build(w, n):
            dg = dgp.tile([128, 128], MMDT, tag="diag")
            nc.vector.tensor_scalar_mul(dg[:], eyeb[:],
                                        adjt[:, w * NV + n:w * NV + n + 1])
            return dg

        def diag_mm(Hp, dg, n, start, stop):
            nc.tensor.matmul(Hp[:], dg[:], gm_sb[n][:],
                             start=start, stop=stop)

        H_next = None

        for v in range(NV):
            # --- HT preparation (chunk-pipelined) ---
            if v == 0:
                hH = None
                HT = None
                R = ps_g.tile([128, HS], F32, tag="g")
                Z = ps_g.tile([128, HS], F32, tag="g")
                NI = ps_g.tile([128, HS], F32, tag="g")
                NH = ps_g.tile([128, HS], F32, tag="g")
                xslot(0, v, R, Z, NI, NH, has_h=False)
            else:
                R = ps_g.tile([128, HS], F32, tag="g")
                Z = ps_g.tile([128, HS], F32, tag="g")
                NI = ps_g.tile([128, HS], F32, tag="g")
                NH = ps_g.tile([128, HS], F32, tag="g")
                xslot(0, v, R, Z, NI, NH, has_h=True)
                hH = sp.tile([128, HS], MMDT, tag="hH")
                nc.scalar.copy(hH[:], H_next[:])
                HT = transpose_chunks(hH, "HT")
                gru_hmms(0, R, Z, NH, HT)

            # fill the GRU1-elementwise PE gap with next-step message terms
            avail = list(range(v))   # gm_0..gm_{v-1} exist already
            half = (len(avail) + 1) // 2
            if v < NV - 1:
                H_next = ps_h.tile([128, HS], F32, tag="H")
                dgs = {i: diag_build(v + 1, i) for i in range(v + 1)}
                for i in avail[:half]:
                    diag_mm(H_next, dgs[i], i, start=(i == 0), stop=False)
            # GRU-p x-slots: banks free up as the GRU-t ew chain consumes
            # them, so these run inside the ew window
            R2 = ps_g.tile([128, HS], F32, tag="g")
            Z2 = ps_g.tile([128, HS], F32, tag="g")
            NI2 = ps_g.tile([128, HS], F32, tag="g")
            NH2 = ps_g.tile([128, HS], F32, tag="g")
            xslot(1, v, R2, Z2, NI2, NH2, has_h=True)
            fill_pe(0 if v < 2 else max(0, FILL1 - 2 * len(avail[:half])))
            hv1 = gru_ew(R, Z, NI, NH, hH)
            hv1T = transpose_chunks(hv1, "hv1T")
            gru_hmms(1, R2, Z2, NH2, hv1T)
            if v < NV - 1:
                for i in avail[half:]:
                    diag_mm(H_next, dgs[i], i, start=(i == 0), stop=False)
            fill_pe(0 if v < 2 else max(0, FILL2 - 2 * len(avail[half:])))
            hv = gru_ew(R2, Z2, NI2, NH2, hv1)
            hvT = transpose_chunks(hv, "hvT")

            if v < NV - 1:
                # gate/mapper for vertex v (feeds future messages)
                gatep = ps_m.tile([128, HS], F32, tag="psm")
                mapp = ps_m.tile([128, HS], F32, tag="psmt")
                vs = slice(v * P, (v + 1) * P)
                nc.tensor.matmul(gatep[:], W["vselp"][0:NV, vs],
                                 W["bgmp"][0:NV, :],
                                 start=True, stop=False, tile_position=(0, 0))
                nc.tensor.matmul(mapp[:], W["vselp"][32:32 + NV, vs],
                                 W["bgmp"][32:32 + NV, :],
                                 start=True, stop=False, tile_position=(32, 0))
                # gate chunks first so the sigmoid can start while the
                # mapper chunks still stream
                for c in range(KC):
                    nc.tensor.matmul(gatep[:], hvT[:, cs(c)],
                                     W["wgm"][:, c * 2 * HS:c * 2 * HS + HS],
                                     start=False, stop=c == KC - 1)
                for c in range(KC):
                    nc.tensor.matmul(mapp[:], hvT[:, cs(c)],
                                     W["wgm"][:, c * 2 * HS + HS:(c + 1) * 2 * HS],
                                     start=False, stop=c == KC - 1)
                gate = sp.tile([128, HS], MMDT, tag="gate")
                nc.scalar.activation(gate[:], gatep[:], AF.Sigmoid)
                gmt = gmc.tile([128, HS], MMDT, tag=f"gm{v}")
                nc.vector.tensor_mul(gmt[:], gate[:], mapp[:])
                gm_sb.append(gmt)
                # last message term for step v+1 (needs gm_v)
                diag_mm(H_next, dgs[v], v, start=(v == 0), stop=True)
                fill_pe(0 if v < 2 else FILL3)
            else:
                # final FC: out = Hg @ Wfc + bfc   (mu | logvar)
                fcp = ps_m.tile([128, 2 * NZ], F32, tag="psm")
                nc.tensor.matmul(fcp[:], W["ones1"][:], W["bfc"][:, :],
                                 start=True, stop=False)
                for c in range(KC):
                    nc.tensor.matmul(fcp[:], hvT[:, cs(c)],
                                     W["wfc"][:, c * 2 * NZ:(c + 1) * 2 * NZ],
                                     start=False, stop=(c == KC - 1))
                fc = sp.tile([128, 2 * NZ], F32, tag="fc")
                nc.scalar.copy(fc[:], fcp[:])
                nc.sync.dma_start(out_ap[:, :], fc[:])


def _host_prep(types, params, adj, gt_wi, gt_wh, gt_bi, gt_bh,
               gp_wi, gp_wh, gp_bi, gp_bh, gate_w, gate_b, mapper_w,
               fc1_w, fc1_b, fc2_w, fc2_b):
    """Pure layout prep: transposes/reshapes/one-hot + per-core sharding."""
    f = np.float32

    def chunked(a):  # [512, X] -> [128, 4*X] with K-chunks side by side
        X = a.shape[1]
        return np.ascontiguousarray(
            a.reshape(KC, 128, X).transpose(1, 0, 2).reshape(128, KC * X)).astype(f)

    b1 = np.concatenate([(gt_bi + gt_bh)[:2 * HS], gt_bi[2 * HS:]])
    b2 = np.concatenate([(gp_bi + gp_bh)[:2 * HS], gp_bi[2 * HS:]])
    w1x = np.concatenate([gt_wi.T, b1[None, :]], 0).astype(f)   # [17, 1536]
    w2x = np.concatenate([gp_wi.T, b2[None, :]], 0).astype(f)   # [33, 1536]

    # packed strip layouts: gate columns of the x-side weights stacked at
    # 32-row offsets so the strip matmuls stream from matching partitions
    w1p = np.zeros((128, HS), f)
    w1p[0:K1] = w1x[:, 0:HS]            # R
    w1p[32:32 + K1] = w1x[:, HS:2 * HS]  # Z
    w1p[64:64 + K1] = w1x[:, 2 * HS:G3]  # NI
    w1p[96] = gt_bh[2 * HS:]             # NH bias
    w2p = np.zeros((128, 2 * HS), f)
    w2p[0:K2, 0:HS] = w2x[:, 0:HS]
    w2p[64:64 + K2, 0:HS] = w2x[:, HS:2 * HS]
    w2p[0:K2, HS:2 * HS] = w2x[:, 2 * HS:G3]
    w2p[96, HS:2 * HS] = gp_bh[2 * HS:]

    bgm = np.stack([np.concatenate([gate_b + gate_w[:, HS + v],
                                    mapper_w[:, HS + v]])
                    for v in range(NV)]).astype(f)               # [NV, 1024]
    bgmp = np.zeros((128, HS), f)
    bgmp[0:NV] = bgm[:, 0:HS]
    bgmp[32:32 + NV] = bgm[:, HS:2 * HS]
    vsel = np.repeat(np.eye(NV, dtype=f), P, axis=1)             # [NV, NV*P]
    vselp = np.zeros((128, NV * P), f)
    vselp[0:NV] = vsel
    vselp[32:32 + NV] = vsel

    shared = {
        "w1p": w1p, "w2p": w2p, "vselp": vselp, "bgmp": bgmp,
        "wht_t": chunked(gt_wh.T.astype(f)),
        "wht_p": chunked(gp_wh.T.astype(f)),
        "wgm": chunked(np.concatenate([gate_w[:, :HS].T, mapper_w[:, :HS].T], 1)),
        "wfc": chunked(np.concatenate([fc1_w.T, fc2_w.T], 1).astype(f)),
        "bfc": np.concatenate([fc1_b, fc2_b])[None, :].astype(f),
        "eyeb": np.eye(128, dtype=f),
        "ones1": np.ones((1, 128), f),
    }
    oh = (types[:, :, None] == np.arange(NVT)[None, None, :]).astype(f)  # [B,NV,NVT]
    in_maps = []
    for c in range(NCORES):
        s = slice(c * P, (c + 1) * P)
        xt = oh[s].transpose(2, 1, 0).reshape(NVT, NV * P)           # [16, NV*P]
        xt1 = np.concatenate([xt, np.ones((1, NV * P), f)], 0)       # [17, NV*P]
        xp = params[s].transpose(2, 1, 0).reshape(FS, NV * P).astype(f)
        xp1 = np.concatenate([xp, np.ones((1, NV * P), f)], 0)       # [33, NV*P]
        xt1p = np.zeros((128, NV * P), f)
        xt1p[0:K1] = xt1
        xt1p[32:32 + K1] = xt1
        xt1p[64:64 + K1] = xt1
        xt1p[96] = 1.0
        xp1p = np.zeros((128, NV * P), f)
        xp1p[0:K2] = xp1
        xp1p[64:64 + K2] = xp1          # row 96 = ones (bias stationary)
        m = dict(shared)
        m["xt1p"] = xt1p
        m["xp1p"] = xp1p
        m["adjt"] = np.ascontiguousarray(adj[s].reshape(P, NV * NV)).astype(f)
        in_maps.append(m)
    return in_maps


_NC_CACHE = {}


def _get_nc():
    key = str(MMDT)
    if key not in _NC_CACHE:
        _NC_CACHE[key] = build_bass()
    return _NC_CACHE[key]


F32_INPUTS = {"adjt"}


def kernel(**inputs):
    np_inputs = {k: np.asarray(v) for k, v in inputs.items()}
    in_maps = _host_prep(**np_inputs)
    npdt = mybir.dt.np(MMDT)
    if npdt != np.float32:
        in_maps = [{k: (v if k in F32_INPUTS else v.astype(npdt))
                    for k, v in m.items()} for m in in_maps]
    nc = _get_nc()
    res = run_bass_kernel_spmd(nc, in_maps, core_ids=list(range(NCORES)),
                               **_RUN_KWARGS)
    out = np.concatenate([res.results[c]["out"] for c in range(NCORES)], 0)
    _LAST_RESULT.clear()
    _LAST_RESULT.append(res)
    return out[:, :NZ], out[:, NZ:]


# test.py can set these to enable tracing / inspect results
_RUN_KWARGS = {}
_LAST_RESULT = []
